# revision 31
# baseline (speedup 1.0000x reference)
"""Causal MHSA prefill kernel for 8 TRN2 NeuronCores.

Sharding: data-parallel over batch (B=2) x tensor-parallel over head groups
(16 heads -> 4 groups of 4). Core c handles batch c//4, heads 4*(c%4)..+3.
Each core computes y_partial[b] = attn_out(heads) @ W_proj[:, cols]^T; the
host sums the 4 partials per batch (the "all-reduce" of the TP hint).

PE operands are bf16 (f32 PSUM accumulation) except the softmax-denominator
rowsums: P tiles are paired, cast to fp8e4 on DVE, and summed with a
DoubleRow fp8 matmul (2 tk-tiles per pass) -- denominator quantization
errors average out across the row, so this is numerically safe, unlike
fp8 anywhere else in the kernel.

Schedule: the attention j-loops (S matmul -> exp on ACT -> rowsum/AV
flush) are rate-limited by ACT's exp, so P1 qkv chains and P3 proj chains
are split into ~4-matmul units woven between j iterations, paced by
estimated PE-ns so exp always has ~1.5-2us of cover.  Diagonal (short-w)
j's run first within each chunk so their exps are covered by the
full-width j's that follow.
"""

import sys

if "/opt/trn_rl_repo" not in sys.path:
    sys.path.insert(0, "/opt/trn_rl_repo")

import numpy as np
import ml_dtypes

import concourse.bacc as bacc
import concourse.tile as tile
from concourse import mybir
from concourse.bass import ts
from concourse.bass_utils import run_bass_kernel_spmd

B, T, D = 2, 2048, 2048
H, DH = 16, 128
HEADS_PER_CORE = 4
N_CORES = 8
NT = T // 128           # 16 token tiles
ND = D // 128           # 16 contraction tiles
NC_CHUNK = T // 512     # 4 tq/t chunks of 512
SCALE = 1.0 / np.sqrt(np.float32(DH))
NEG = -1.0e30

F32 = mybir.dt.float32
BF16 = mybir.dt.bfloat16
FP8 = mybir.dt.float8e4
DR = mybir.MatmulPerfMode.DoubleRow
EXP = mybir.ActivationFunctionType.Exp
COPY = mybir.ActivationFunctionType.Copy

_compiled = None


def _build():
    nc = bacc.Bacc("TRN2", target_bir_lowering=False, debug=False,
                   num_devices=N_CORES)

    xT = nc.dram_tensor("xT", [D, T], BF16, kind="ExternalInput")
    # per head-pair blocks of W_qkv^T: cols = [q(2x128) | k(2x128) | v(2x128)]
    wT = nc.dram_tensor("wT", [2, D, 768], BF16, kind="ExternalInput")
    wpT = nc.dram_tensor("wpT", [HEADS_PER_CORE * DH, D], BF16,
                         kind="ExternalInput")
    mask = nc.dram_tensor("mask", [128, 128], F32, kind="ExternalInput")
    ones = nc.dram_tensor("ones", [128, 128], BF16, kind="ExternalInput")
    # DoubleRow stationary: [Ki, 2, 16] (pair-dim step must be 16B-aligned);
    # only column 0 is ones, so only out-partition 0 carries the sum.
    ones8 = nc.dram_tensor("ones8", [128, 2, 16], FP8, kind="ExternalInput")
    y = nc.dram_tensor("y", [T, D], BF16, kind="ExternalOutput")

    xT_r = xT.ap().rearrange("(n p) t -> p n t", p=128)
    qk_tags = [["q0_h0", "q1_h0", "k0_h0", "k1_h0"],
               ["q0_h1", "q1_h1", "k0_h1", "k1_h1"]]

    with tile.TileContext(nc) as tc:
        with (
            tc.tile_pool(name="persist", bufs=1) as persist,
            tc.tile_pool(name="wpool", bufs=2) as wpool,
            tc.tile_pool(name="xt", bufs=3) as xtp,
            tc.tile_pool(name="work", bufs=2) as work,
            tc.tile_pool(name="ybuf", bufs=6) as ybuf,
            tc.tile_pool(name="ps2", bufs=3, space="PSUM") as ps2,
            tc.tile_pool(name="ps1", bufs=2, space="PSUM") as ps1,
        ):
            qk = [None, None]   # per hp: [q0, q1, k0, k1] tiles [128, T]
            v_sb = [None, None]
            w_sb = [None, None]
            attnT = [persist.tile([128, T], BF16, tag=f"attnT{i}",
                                  name=f"attnT{i}")
                     for i in range(HEADS_PER_CORE)]

            state = {"tail": None}

            def emit_tail(tail):
                rs_inv, oT, t, ps_o = tail
                bc = work.tile([128, 512], F32, tag="bc", bufs=2, name="bc")
                nc.gpsimd.partition_broadcast(bc, rs_inv, channels=128)
                nc.vector.tensor_mul(oT[:, ts(t, 512)], ps_o, bc)

            def maybe_tail():
                if state["tail"] is not None:
                    emit_tail(state["tail"])
                    state["tail"] = None

            def dma_w(hp):
                # 4-tile chunks, not per-tile: fewer DMA-complete semaphores
                # for the matmul chains to wait on (waits after the first are
                # elided by engine program order).
                w_sb[hp] = wpool.tile([128, ND, 768], BF16, tag="w",
                                      name=f"w_h{hp}")
                wT_r = wT.ap()[hp].rearrange("(n p) e -> p n e", p=128)
                for n0 in range(0, ND, 4):
                    nc.gpsimd.dma_start(out=w_sb[hp][:, n0:n0 + 4, :],
                                        in_=wT_r[:, n0:n0 + 4, :])

            def dma_x(tci, queue=None):
                q = queue or nc.gpsimd
                xt_a = xtp.tile([128, 8, 512], BF16, tag="xta", name="xt_a")
                xt_b = xtp.tile([128, 8, 512], BF16, tag="xtb", name="xt_b")
                q.dma_start(out=xt_a, in_=xT_r[:, 0:8, ts(tci, 512)])
                q.dma_start(out=xt_b, in_=xT_r[:, 8:16, ts(tci, 512)])
                return xt_a, xt_b

            def p1_alloc(hp):
                qk[hp] = [persist.tile([128, T], BF16, tag=t2,
                                       name=f"{t2}")
                          for t2 in qk_tags[hp]]
                v_sb[hp] = persist.tile([128, NT, 256], BF16,
                                        tag=f"v_h{hp}", name=f"v_h{hp}")

            QK_SEG_NS = 4 * 512 / 2.4
            V_SEG_NS = 4 * 256 / 2.4
            PROJ_NS = 4 * 512 / 2.4 + 150

            def p1_units(hp, tci, xts):
                """qkv for 512 tokens as (est_ns, emit_fn) units.

                4 feature-major q/k chains + 4 token-major v chains, each
                split into 4-matmul segments; the last segment appends the
                PSUM evacuation (qk -> ACT copy, v -> DVE copy)."""
                xt_a, xt_b = xts

                def xrhs(n):
                    return xt_a[:, n, :] if n < 8 else xt_b[:, n - 8, :]

                units = []
                for et in range(4):
                    box = {}

                    def mk(et, box, s0):
                        def seg():
                            if s0 == 0:
                                box["ps"] = ps2.tile([128, 512], F32,
                                                     tag="s", bufs=3,
                                                     name="qk_ps")
                            for n in range(s0, s0 + 4):
                                nc.tensor.matmul(
                                    box["ps"], w_sb[hp][:, n, ts(et, 128)],
                                    xrhs(n), start=(n == 0),
                                    stop=(n == ND - 1))
                            if s0 == 12:
                                nc.scalar.activation(
                                    qk[hp][et][:, ts(tci, 512)], box["ps"],
                                    COPY)
                        return seg
                    for s0 in range(0, ND, 4):
                        units.append((QK_SEG_NS, mk(et, box, s0)))
                for tt in range(4):
                    box = {}

                    def mkv(tt, box, s0):
                        def seg():
                            if s0 == 0:
                                box["ps"] = ps2.tile([128, 256], F32,
                                                     tag="o", bufs=3,
                                                     name="v_ps")
                            for n in range(s0, s0 + 4):
                                lhsT = (xt_a[:, n, ts(tt, 128)] if n < 8
                                        else xt_b[:, n - 8, ts(tt, 128)])
                                nc.tensor.matmul(
                                    box["ps"], lhsT, w_sb[hp][:, n, 512:768],
                                    start=(n == 0), stop=(n == ND - 1))
                            if s0 == 12:
                                nc.vector.tensor_copy(
                                    v_sb[hp][:, tci * 4 + tt, :], box["ps"])
                        return seg
                    for s0 in range(0, ND, 4):
                        units.append((V_SEG_NS, mkv(tt, box, s0)))
                return units

            mask_sb = persist.tile([128, 128], F32, tag="mask")
            # host "ones" input has col 0 = 1, cols 1..15 = 0:
            # ones_col is the plain rowsum stationary; ones16 starts the
            # full 16-partition ps_rs region the DoubleRow pairs write to.
            ones_sb = persist.tile([128, 16], BF16, tag="ones_sb")
            ones_col = ones_sb[:, 0:1]
            ones_pair = persist.tile([128, 2, 16], FP8, tag="ones_pair")

            def p2_chunk(hp, t, i, weave=None):
                """Causal attention for head i of pair hp, tq chunk t.

                j order: the 4 diagonal (masked, short-w) tiles first, then
                the full-width tiles 0..4t-1 processed in fp8-rowsum pairs.
                """
                qT, kT = qk[hp][i], qk[hp][2 + i]
                oT = attnT[hp * 2 + i]
                ps_o = ps2.tile([128, 512], F32, tag="o", bufs=3,
                                name="ps_o")
                # row 0 = rowsums; rows 1..15 are DoubleRow zero-column
                # fill, never read
                ps_rs = ps1.tile([16, 512], F32, tag="rs", bufs=2,
                                 name="ps_rs")
                order = list(range(4 * t, 4 * t + 4)) + list(range(4 * t))
                n_pairs = 2 * t

                pend_q = []     # AV (and diagonal-rowsum) flushes
                pair_pend = []  # fp8 rowsum-pair flushes

                def flush(pend):
                    p_ap, off, w, j, diag, first, last = pend
                    if diag:   # diagonal j: bf16 rowsum (not in any pair)
                        # when DR pairs follow, the first rowsum uses the
                        # 16-wide stationary so start=True opens all 16
                        # ps_rs partitions; the last pair's stop closes
                        # the same region
                        if first and n_pairs > 0:
                            nc.tensor.matmul(ps_rs[:, off:off + w], ones_sb,
                                             p_ap[:, :w], start=True,
                                             stop=(last and n_pairs == 0))
                        else:
                            nc.tensor.matmul(ps_rs[0:1, off:off + w],
                                             ones_col, p_ap[:, :w],
                                             start=first,
                                             stop=(last and n_pairs == 0))
                    nc.tensor.matmul(ps_o[:, off:off + w],
                                     v_sb[hp][:, j, ts(i, 128)],
                                     p_ap[:, :w], start=first, stop=last)

                def flush_pair(p8):
                    nc.tensor.matmul(ps_rs, ones_pair, p8,
                                     start=False, stop=p8 is pair_pend_last,
                                     perf_mode=DR)

                pair_pend_last = None
                p_pair = None
                for idx, j in enumerate(order):
                    diag = j >= 4 * t
                    off = (j - 4 * t) * 128 if diag else 0
                    w = 512 - off
                    ps_s = ps2.tile([128, 512], F32, tag="s", bufs=3,
                                    name="ps_s")
                    nc.tensor.matmul(
                        ps_s[:, :w], kT[:, ts(j, 128)],
                        qT[:, t * 512 + off:(t + 1) * 512],
                        start=True, stop=True)
                    if idx == 2:
                        maybe_tail()
                    if len(pend_q) >= 2:
                        flush(pend_q.pop(0))
                    if len(pair_pend) >= 3:
                        flush_pair(pair_pend.pop(0))
                    if weave is not None:
                        weave(2.2 * w / 2.4 + 250)
                    if diag:
                        nc.vector.tensor_add(ps_s[:, 0:128], ps_s[:, 0:128],
                                             mask_sb)
                        p_ap = work.tile([128, 512], BF16, tag="P", bufs=3,
                                         name="p_sb")
                    else:
                        half = (idx - 4) % 2
                        if half == 0:
                            p_pair = work.tile([128, 2, 512], BF16,
                                               tag="P2", bufs=3,
                                               name="p_pair")
                        p_ap = p_pair[:, half, :]
                    nc.scalar.activation(p_ap[:, :w], ps_s[:, :w], EXP,
                                         scale=float(SCALE))
                    if not diag and (idx - 4) % 2 == 1:
                        p8 = work.tile([128, 2, 512], FP8, tag="P8",
                                       bufs=4, name="p8")
                        nc.vector.tensor_copy(p8, p_pair)
                        pair_pend.append(p8)
                        if idx == len(order) - 1:
                            pair_pend_last = p8
                    pend_q.append((p_ap, off, w, j, diag, idx == 0,
                                   idx == len(order) - 1))
                for p in pend_q:
                    if weave is not None:
                        weave(900)
                    flush(p)
                for p8 in pair_pend:
                    if weave is not None:
                        weave(500)
                    flush_pair(p8)
                rs_inv = work.tile([1, 512], F32, tag="rsi", bufs=2,
                                   name="rs_inv")
                with nc.allow_low_precision(
                        reason="approx reciprocal of softmax denom"):
                    nc.vector.reciprocal_approx_fast(out=rs_inv,
                                                     in_=ps_rs[0:1, :])
                state["tail"] = (rs_inv, oT, t, ps_o)

            wp = [None] * 4

            def dma_wp():
                wpT_ap = wpT.ap()
                for e in range(4):
                    wp[e] = persist.tile([128, D], BF16, tag=qk_tags[0][e],
                                         name=f"wp{e}")
                    nc.gpsimd.dma_start(out=wp[e], in_=wpT_ap[ts(e, 128), :])

            def p3_units(tb, evac_act=False):
                """Proj for token tiles 4*tb..4*tb+3, two units per chain.

                Woven batches evacuate on DVE (ACT is the exp engine and is
                the binding resource during interleave B); the final,
                exp-free batch uses ACT (evac_act)."""
                units = []
                for mi in range(4):
                    for nck in range(NC_CHUNK):
                        box = {}

                        def mk(mi=mi, nck=nck, box=box, head=True):
                            def unit():
                                m = tb * 4 + mi
                                k = m * 4 + nck
                                if head:
                                    tg = "s" if k % 2 == 0 else "o"
                                    box["ps"] = ps2.tile(
                                        [128, 512], F32, tag=tg, bufs=3,
                                        name="proj_ps")
                                for e in ((0, 1) if head else (2, 3)):
                                    nc.tensor.matmul(
                                        box["ps"], attnT[e][:, ts(m, 128)],
                                        wp[e][:, ts(nck, 512)],
                                        start=(e == 0), stop=(e == 3))
                                if not head:
                                    y_sb = ybuf.tile([128, 512], BF16,
                                                     tag="y", bufs=6,
                                                     name="y_sb")
                                    if evac_act:
                                        nc.scalar.activation(y_sb,
                                                             box["ps"], COPY)
                                    else:
                                        nc.vector.tensor_copy(y_sb,
                                                              box["ps"])
                                    nc.sync.dma_start(
                                        out=y.ap()[ts(m, 128),
                                                   ts(nck, 512)],
                                        in_=y_sb)
                            return unit
                        units.append((PROJ_NS / 2, mk(head=True)))
                        units.append((PROJ_NS / 2, mk(head=False)))
                return units

            def make_weaver(units):
                """Pace unit emission by estimated PE-ns fractions."""
                total_p2 = {"ns": 0.0}
                total_units = sum(u[0] for u in units)
                st = {"done": 0, "done_ns": 0.0, "p2_ns": 0.0}

                def weave(p2_ns):
                    st["p2_ns"] += p2_ns
                    if total_p2["ns"] <= 0:
                        return
                    tgt = total_units * st["p2_ns"] / total_p2["ns"]
                    while st["done"] < len(units) and st["done_ns"] < tgt:
                        ns, fn = units[st["done"]]
                        fn()
                        st["done"] += 1
                        st["done_ns"] += ns

                def drain():
                    while st["done"] < len(units):
                        units[st["done"]][1]()
                        st["done"] += 1
                weave.drain = drain
                weave.set_total = lambda v: total_p2.__setitem__("ns", v)
                return weave

            def p2_pair_ns(t):
                """Approx PE-ns of one head's p2_chunk weave() calls."""
                diag = sum(2.2 * (512 - r * 128) / 2.4 + 250
                           for r in range(4))
                full = 4 * t * (2.2 * 512 / 2.4 + 250)
                return diag + full + 2 * 900 + (3 * 500 if t else 0)

            # ---- emission schedule ----
            # Prologue: all bulk loads on ONE queue in strict first-need
            # order (DMA engines drain roughly in issue order, so a later-
            # needed bulk transfer issued early delays an earlier-needed
            # one).  q cols gate the first chain (~1MB), then x halves,
            # then k cols (needed 2 chains in), then v cols.
            w_sb[0] = wpool.tile([128, ND, 768], BF16, tag="w", name="w_h0")
            wT_r0 = wT.ap()[0].rearrange("(n p) e -> p n e", p=128)
            xt_a = xtp.tile([128, 8, 512], BF16, tag="xta", name="xt_a")
            xt_b = xtp.tile([128, 8, 512], BF16, tag="xtb", name="xt_b")
            nc.gpsimd.dma_start(out=w_sb[0][:, :, 0:128],
                                in_=wT_r0[:, :, 0:128])
            nc.gpsimd.dma_start(out=xt_a[:, 0:4, :],
                                in_=xT_r[:, 0:4, 0:512])
            nc.gpsimd.dma_start(out=w_sb[0][:, :, 128:256],
                                in_=wT_r0[:, :, 128:256])
            nc.gpsimd.dma_start(out=xt_a[:, 4:8, :],
                                in_=xT_r[:, 4:8, 0:512])
            nc.gpsimd.dma_start(out=xt_b, in_=xT_r[:, 8:16, 0:512])
            nc.gpsimd.dma_start(out=w_sb[0][:, :, 256:512],
                                in_=wT_r0[:, :, 256:512])
            nc.gpsimd.dma_start(out=w_sb[0][:, :, 512:768],
                                in_=wT_r0[:, :, 512:768])
            nc.sync.dma_start(out=mask_sb, in_=mask.ap())
            nc.sync.dma_start(out=ones_sb, in_=ones.ap()[:, 0:16])
            nc.sync.dma_start(out=ones_pair, in_=ones8.ap())
            xts0 = (xt_a, xt_b)
            p1_alloc(0)

            # P1(hp0): pure qkv streaming for head pair 0
            xts_a0 = None
            for tci in range(NC_CHUNK):
                xts = xts0 if tci == 0 else dma_x(tci)
                if tci == 2:
                    dma_w(1)      # stream hp1 weights under hp0 compute
                if tci == 3:
                    # x chunk 0 for interleave A: woven p1 units consume it
                    # almost immediately at A start, so land it under P1's
                    # last chunk.
                    xts_a0 = dma_x(0)
                for _, u in p1_units(0, tci, xts):
                    u()

            # Interleave A: P2(hp0) j-steps woven with P1(hp1) units.
            # The last chunk's weaver is not drained: its leftover units
            # spill into interleave B's t=0 (which otherwise has no woven
            # PE work and stalls on ACT).
            p1_alloc(1)
            xts = xts_a0
            carry = None
            for t in range(NC_CHUNK):
                weave = make_weaver(p1_units(1, t, xts))
                if t + 1 < NC_CHUNK:
                    weave.set_total(2 * p2_pair_ns(t))
                else:
                    weave.set_total(2 * (p2_pair_ns(t) + p2_pair_ns(0)))
                p2_chunk(0, t, 0, weave)
                if t + 1 < NC_CHUNK:
                    xts = dma_x(t + 1)
                p2_chunk(0, t, 1, weave)
                if t + 1 < NC_CHUNK:
                    weave.drain()
                else:
                    carry = weave

            # Interleave B: P2(hp1) j-steps woven with P3 proj units.
            # The pending tail must flush before batch t-1's proj units may
            # read attnT, so flush it before each chunk's first j-loop.
            dma_wp()
            for t in range(NC_CHUNK):
                maybe_tail()
                if t >= 1:
                    if carry is not None:
                        carry.drain()
                        carry = None
                    weave = make_weaver(p3_units(t - 1))
                    weave.set_total(2 * p2_pair_ns(t))
                else:
                    weave = carry   # leftover P1(hp1) units from A
                p2_chunk(1, t, 0, weave)
                p2_chunk(1, t, 1, weave)
                if t >= 1:
                    weave.drain()
            maybe_tail()
            for _, u in p3_units(NC_CHUNK - 1, evac_act=True):
                u()

    nc.compile()
    return nc


def _get_compiled():
    global _compiled
    if _compiled is None:
        _compiled = _build()
    return _compiled


def ones8_host():
    o8 = np.zeros((128, 2, 16), dtype=ml_dtypes.float8_e4m3)
    o8[:, :, 0] = 1.0
    return o8


def _shard_inputs(x, W_qkv, W_proj):
    """Build the 8 per-core input maps (host-side transposes/slices)."""
    bf16 = ml_dtypes.bfloat16
    x = np.asarray(x, dtype=np.float32)
    W_qkv = np.asarray(W_qkv, dtype=np.float32)
    W_proj = np.asarray(W_proj, dtype=np.float32)

    mask = np.where(np.arange(128)[None, :] >= np.arange(128)[:, None],
                    np.float32(0.0), np.float32(NEG))  # [tk, tq]

    in_maps = []
    for c in range(N_CORES):
        b, g = divmod(c, HEADS_PER_CORE)
        xT = np.ascontiguousarray(x[b].T).astype(bf16)
        wt = np.empty((2, D, 768), dtype=bf16)
        for hp in range(2):
            rows = []
            for blk in range(3):  # q, k, v row blocks of W_qkv
                h0 = (4 * g + 2 * hp) * DH
                rows.append(W_qkv[blk * D + h0: blk * D + h0 + 2 * DH])
            wt[hp] = np.concatenate(rows, axis=0).T.astype(bf16)
        cols = slice(4 * g * DH, 4 * g * DH + HEADS_PER_CORE * DH)
        wpT = np.ascontiguousarray(W_proj[:, cols].T).astype(bf16)
        ones_arr = np.zeros((128, 128), dtype=bf16)
        ones_arr[:, 0] = 1.0
        in_maps.append({"xT": xT, "wT": wt, "wpT": wpT, "mask": mask,
                        "ones": ones_arr, "ones8": ones8_host()})
    return in_maps


def kernel(x, W_qkv, W_proj, step, trace=False, trace_cores=None):
    nc = _get_compiled()
    in_maps = _shard_inputs(x, W_qkv, W_proj)
    res = run_bass_kernel_spmd(nc, in_maps, list(range(N_CORES)),
                               trace=trace, trace_cores=trace_cores)
    y = np.zeros((B, T, D), dtype=np.float32)
    for c in range(N_CORES):
        y[c // HEADS_PER_CORE] += np.asarray(res.results[c]["y"],
                                             dtype=np.float32)
    kernel.last_exec_time_ns = res.exec_time_ns
    return y


# revision 38
# speedup vs baseline: 1.0027x; 1.0027x over previous
"""Causal MHSA prefill kernel for 8 TRN2 NeuronCores.

Sharding: data-parallel over batch (B=2) x tensor-parallel over head groups
(16 heads -> 4 groups of 4). Core c handles batch c//4, heads 4*(c%4)..+3.
Each core computes y_partial[b] = attn_out(heads) @ W_proj[:, cols]^T; the
host sums the 4 partials per batch (the "all-reduce" of the TP hint).

PE operands are bf16 (f32 PSUM accumulation) except the softmax-denominator
rowsums: P tiles are paired, cast to fp8e4 on DVE, and summed with a
DoubleRow fp8 matmul (2 tk-tiles per pass) -- denominator quantization
errors average out across the row, so this is numerically safe, unlike
fp8 anywhere else in the kernel.

Schedule: the attention j-loops (S matmul -> exp on ACT -> rowsum/AV
flush) are rate-limited by ACT's exp, so P1 qkv chains and P3 proj chains
are split into ~4-matmul units woven between j iterations, paced by
estimated PE-ns so exp always has ~1.5-2us of cover.  Diagonal (short-w)
j's run first within each chunk so their exps are covered by the
full-width j's that follow.
"""

import sys

if "/opt/trn_rl_repo" not in sys.path:
    sys.path.insert(0, "/opt/trn_rl_repo")

import numpy as np
import ml_dtypes

import concourse.bacc as bacc
import concourse.tile as tile
from concourse import mybir
from concourse.bass import ts
from concourse.bass_utils import run_bass_kernel_spmd

B, T, D = 2, 2048, 2048
H, DH = 16, 128
HEADS_PER_CORE = 4
N_CORES = 8
NT = T // 128           # 16 token tiles
ND = D // 128           # 16 contraction tiles
NC_CHUNK = T // 512     # 4 tq/t chunks of 512
SCALE = 1.0 / np.sqrt(np.float32(DH))
NEG = -1.0e30

F32 = mybir.dt.float32
BF16 = mybir.dt.bfloat16
FP8 = mybir.dt.float8e4
DR = mybir.MatmulPerfMode.DoubleRow
EXP = mybir.ActivationFunctionType.Exp
COPY = mybir.ActivationFunctionType.Copy

_compiled = None


def _build():
    nc = bacc.Bacc("TRN2", target_bir_lowering=False, debug=False,
                   num_devices=N_CORES)

    xT = nc.dram_tensor("xT", [D, T], BF16, kind="ExternalInput")
    # per head-pair blocks of W_qkv^T: cols = [q(2x128) | k(2x128) | v(2x128)]
    wT = nc.dram_tensor("wT", [2, D, 768], BF16, kind="ExternalInput")
    wpT = nc.dram_tensor("wpT", [HEADS_PER_CORE * DH, D], BF16,
                         kind="ExternalInput")
    mask = nc.dram_tensor("mask", [128, 128], F32, kind="ExternalInput")
    ones = nc.dram_tensor("ones", [128, 128], BF16, kind="ExternalInput")
    # DoubleRow stationary: [Ki, 2, 16] (pair-dim step must be 16B-aligned);
    # only column 0 is ones, so only out-partition 0 carries the sum.
    ones8 = nc.dram_tensor("ones8", [128, 2, 16], FP8, kind="ExternalInput")
    y = nc.dram_tensor("y", [T, D], BF16, kind="ExternalOutput")

    xT_r = xT.ap().rearrange("(n p) t -> p n t", p=128)
    qk_tags = [["q0_h0", "q1_h0", "k0_h0", "k1_h0"],
               ["q0_h1", "q1_h1", "k0_h1", "k1_h1"]]

    with tile.TileContext(nc) as tc:
        with (
            tc.tile_pool(name="persist", bufs=1) as persist,
            tc.tile_pool(name="wpool", bufs=2) as wpool,
            tc.tile_pool(name="xt", bufs=3) as xtp,
            tc.tile_pool(name="work", bufs=2) as work,
            tc.tile_pool(name="ybuf", bufs=6) as ybuf,
            tc.tile_pool(name="ps2", bufs=3, space="PSUM") as ps2,
            tc.tile_pool(name="ps1", bufs=2, space="PSUM") as ps1,
        ):
            qk = [None, None]   # per hp: [q0, q1, k0, k1] tiles [128, T]
            v_sb = [None, None]
            w_sb = [None, None]
            attnT = [persist.tile([128, T], BF16, tag=f"attnT{i}",
                                  name=f"attnT{i}")
                     for i in range(HEADS_PER_CORE)]

            def emit_tail(rs_inv, oT, t, ps_o):
                bc = work.tile([128, 512], F32, tag="bc", bufs=2, name="bc")
                nc.gpsimd.partition_broadcast(bc, rs_inv, channels=128)
                nc.vector.tensor_mul(oT[:, ts(t, 512)], ps_o, bc)

            def dma_w(hp):
                # 4-tile chunks, not per-tile: fewer DMA-complete semaphores
                # for the matmul chains to wait on (waits after the first are
                # elided by engine program order).
                w_sb[hp] = wpool.tile([128, ND, 768], BF16, tag="w",
                                      name=f"w_h{hp}")
                wT_r = wT.ap()[hp].rearrange("(n p) e -> p n e", p=128)
                for n0 in range(0, ND, 4):
                    nc.gpsimd.dma_start(out=w_sb[hp][:, n0:n0 + 4, :],
                                        in_=wT_r[:, n0:n0 + 4, :])

            def dma_x(tci, queue=None):
                q = queue or nc.gpsimd
                xt_a = xtp.tile([128, 8, 512], BF16, tag="xta", name="xt_a")
                xt_b = xtp.tile([128, 8, 512], BF16, tag="xtb", name="xt_b")
                q.dma_start(out=xt_a, in_=xT_r[:, 0:8, ts(tci, 512)])
                q.dma_start(out=xt_b, in_=xT_r[:, 8:16, ts(tci, 512)])
                return xt_a, xt_b

            def p1_alloc(hp):
                qk[hp] = [persist.tile([128, T], BF16, tag=t2,
                                       name=f"{t2}")
                          for t2 in qk_tags[hp]]
                v_sb[hp] = persist.tile([128, NT, 256], BF16,
                                        tag=f"v_h{hp}", name=f"v_h{hp}")

            QK_SEG_NS = 4 * 512 / 2.4
            V_SEG_NS = 4 * 256 / 2.4
            PROJ_NS = 4 * 512 / 2.4 + 150

            def p1_units(hp, tci, xts):
                """qkv for 512 tokens as (est_ns, emit_fn) units.

                4 feature-major q/k chains + 4 token-major v chains, each
                split into 4-matmul segments; the last segment appends the
                PSUM evacuation (qk -> ACT copy, v -> DVE copy)."""
                xt_a, xt_b = xts

                def xrhs(n):
                    return xt_a[:, n, :] if n < 8 else xt_b[:, n - 8, :]

                units = []
                for et in range(4):
                    box = {}

                    def mk(et, box, s0):
                        def seg():
                            if s0 == 0:
                                box["ps"] = ps2.tile([128, 512], F32,
                                                     tag="s", bufs=3,
                                                     name="qk_ps")
                            for n in range(s0, s0 + 4):
                                nc.tensor.matmul(
                                    box["ps"], w_sb[hp][:, n, ts(et, 128)],
                                    xrhs(n), start=(n == 0),
                                    stop=(n == ND - 1))
                            if s0 == 12:
                                nc.scalar.activation(
                                    qk[hp][et][:, ts(tci, 512)], box["ps"],
                                    COPY)
                        return seg
                    for s0 in range(0, ND, 4):
                        units.append((QK_SEG_NS, mk(et, box, s0)))
                for tt in range(4):
                    box = {}

                    def mkv(tt, box, s0):
                        def seg():
                            if s0 == 0:
                                box["ps"] = ps2.tile([128, 256], F32,
                                                     tag="o", bufs=3,
                                                     name="v_ps")
                            for n in range(s0, s0 + 4):
                                lhsT = (xt_a[:, n, ts(tt, 128)] if n < 8
                                        else xt_b[:, n - 8, ts(tt, 128)])
                                nc.tensor.matmul(
                                    box["ps"], lhsT, w_sb[hp][:, n, 512:768],
                                    start=(n == 0), stop=(n == ND - 1))
                            if s0 == 12:
                                nc.vector.tensor_copy(
                                    v_sb[hp][:, tci * 4 + tt, :], box["ps"])
                        return seg
                    for s0 in range(0, ND, 4):
                        units.append((V_SEG_NS, mkv(tt, box, s0)))
                return units

            mask_sb = persist.tile([128, 128], F32, tag="mask")
            # host "ones" input has col 0 = 1, cols 1..15 = 0:
            # ones_col is the plain rowsum stationary; ones16 starts the
            # full 16-partition ps_rs region the DoubleRow pairs write to.
            ones_sb = persist.tile([128, 16], BF16, tag="ones_sb")
            ones_col = ones_sb[:, 0:1]
            ones_pair = persist.tile([128, 2, 16], FP8, tag="ones_pair")

            def p2_chunk(hp, t, i, weave=None):
                """Causal attention for head i of pair hp, tq chunk t.

                j order: the 4 diagonal (masked, short-w) tiles first, then
                the full-width tiles 0..4t-1 processed in fp8-rowsum pairs.
                """
                qT, kT = qk[hp][i], qk[hp][2 + i]
                oT = attnT[hp * 2 + i]
                ps_o = ps2.tile([128, 512], F32, tag="o", bufs=3,
                                name="ps_o")
                # row 0 = rowsums; rows 1..15 are DoubleRow zero-column
                # fill, never read
                ps_rs = ps1.tile([16, 512], F32, tag="rs", bufs=2,
                                 name="ps_rs")
                order = list(range(4 * t, 4 * t + 4)) + list(range(4 * t))
                n_pairs = 2 * t

                pend_q = []     # AV (and diagonal-rowsum) flushes
                pair_pend = []  # fp8 rowsum-pair flushes

                def flush(pend):
                    p_ap, off, w, j, diag, first, last = pend
                    if diag:   # diagonal j: bf16 rowsum (not in any pair)
                        # when DR pairs follow, the first rowsum uses the
                        # 16-wide stationary so start=True opens all 16
                        # ps_rs partitions; the last pair's stop closes
                        # the same region
                        if first and n_pairs > 0:
                            nc.tensor.matmul(ps_rs[:, off:off + w], ones_sb,
                                             p_ap[:, :w], start=True,
                                             stop=(last and n_pairs == 0))
                        else:
                            nc.tensor.matmul(ps_rs[0:1, off:off + w],
                                             ones_col, p_ap[:, :w],
                                             start=first,
                                             stop=(last and n_pairs == 0))
                    nc.tensor.matmul(ps_o[:, off:off + w],
                                     v_sb[hp][:, j, ts(i, 128)],
                                     p_ap[:, :w], start=first, stop=last)

                def flush_pair(p8):
                    nc.tensor.matmul(ps_rs, ones_pair, p8,
                                     start=False, stop=p8 is pair_pend_last,
                                     perf_mode=DR)

                pair_pend_last = None
                p_pair = None
                for idx, j in enumerate(order):
                    diag = j >= 4 * t
                    off = (j - 4 * t) * 128 if diag else 0
                    w = 512 - off
                    ps_s = ps2.tile([128, 512], F32, tag="s", bufs=3,
                                    name="ps_s")
                    nc.tensor.matmul(
                        ps_s[:, :w], kT[:, ts(j, 128)],
                        qT[:, t * 512 + off:(t + 1) * 512],
                        start=True, stop=True)
                    if len(pend_q) >= 2:
                        flush(pend_q.pop(0))
                    if len(pair_pend) >= 3:
                        flush_pair(pair_pend.pop(0))
                    if weave is not None:
                        weave(2.2 * w / 2.4 + 250)
                    if diag:
                        nc.vector.tensor_add(ps_s[:, 0:128], ps_s[:, 0:128],
                                             mask_sb)
                        p_ap = work.tile([128, 512], BF16, tag="P", bufs=3,
                                         name="p_sb")
                    else:
                        half = (idx - 4) % 2
                        if half == 0:
                            p_pair = work.tile([128, 2, 512], BF16,
                                               tag="P2", bufs=3,
                                               name="p_pair")
                        p_ap = p_pair[:, half, :]
                    nc.scalar.activation(p_ap[:, :w], ps_s[:, :w], EXP,
                                         scale=float(SCALE))
                    if not diag and (idx - 4) % 2 == 1:
                        p8 = work.tile([128, 2, 512], FP8, tag="P8",
                                       bufs=4, name="p8")
                        nc.vector.tensor_copy(p8, p_pair)
                        pair_pend.append(p8)
                        if idx == len(order) - 1:
                            pair_pend_last = p8
                    pend_q.append((p_ap, off, w, j, diag, idx == 0,
                                   idx == len(order) - 1))
                for p in pend_q:
                    if weave is not None:
                        weave(450)
                    flush(p)
                for p8 in pair_pend:
                    if weave is not None:
                        weave(250)
                    flush_pair(p8)
                rs_inv = work.tile([1, 512], F32, tag="rsi", bufs=2,
                                   name="rs_inv")
                with nc.allow_low_precision(
                        reason="approx reciprocal of softmax denom"):
                    nc.vector.reciprocal_approx_fast(out=rs_inv,
                                                     in_=ps_rs[0:1, :])
                # normalize immediately: the sooner attnT lands, the sooner
                # woven proj units may read it and ps_o's bank recycles
                emit_tail(rs_inv, oT, t, ps_o)

            wp = [None] * 4

            def dma_wp():
                wpT_ap = wpT.ap()
                for e in range(4):
                    wp[e] = persist.tile([128, D], BF16, tag=qk_tags[0][e],
                                         name=f"wp{e}")
                    nc.gpsimd.dma_start(out=wp[e], in_=wpT_ap[ts(e, 128), :])

            def p3_units(tb, evac_act=False):
                """Proj for token tiles 4*tb..4*tb+3, two units per chain.

                Woven batches evacuate on DVE (ACT is the exp engine and is
                the binding resource during interleave B); the final,
                exp-free batch uses ACT (evac_act)."""
                units = []
                for mi in range(4):
                    for nck in range(NC_CHUNK):
                        box = {}

                        def mk(mi=mi, nck=nck, box=box, head=True):
                            def unit():
                                m = tb * 4 + mi
                                k = m * 4 + nck
                                if head:
                                    tg = "s" if k % 2 == 0 else "o"
                                    box["ps"] = ps2.tile(
                                        [128, 512], F32, tag=tg, bufs=3,
                                        name="proj_ps")
                                for e in ((0, 1) if head else (2, 3)):
                                    nc.tensor.matmul(
                                        box["ps"], attnT[e][:, ts(m, 128)],
                                        wp[e][:, ts(nck, 512)],
                                        start=(e == 0), stop=(e == 3))
                                if not head:
                                    y_sb = ybuf.tile([128, 512], BF16,
                                                     tag="y", bufs=6,
                                                     name="y_sb")
                                    if evac_act:
                                        nc.scalar.activation(y_sb,
                                                             box["ps"], COPY)
                                    else:
                                        nc.vector.tensor_copy(y_sb,
                                                              box["ps"])
                                    nc.sync.dma_start(
                                        out=y.ap()[ts(m, 128),
                                                   ts(nck, 512)],
                                        in_=y_sb)
                            return unit
                        units.append((PROJ_NS / 2, mk(head=True)))
                        units.append((PROJ_NS / 2, mk(head=False)))
                return units

            def make_weaver(units):
                """Pace unit emission by estimated PE-ns fractions."""
                total_p2 = {"ns": 0.0}
                total_units = sum(u[0] for u in units)
                st = {"done": 0, "done_ns": 0.0, "p2_ns": 0.0}

                def weave(p2_ns):
                    st["p2_ns"] += p2_ns
                    if total_p2["ns"] <= 0:
                        return
                    tgt = total_units * st["p2_ns"] / total_p2["ns"]
                    while st["done"] < len(units) and st["done_ns"] < tgt:
                        ns, fn = units[st["done"]]
                        fn()
                        st["done"] += 1
                        st["done_ns"] += ns

                def drain():
                    while st["done"] < len(units):
                        units[st["done"]][1]()
                        st["done"] += 1
                weave.drain = drain
                weave.set_total = lambda v: total_p2.__setitem__("ns", v)
                return weave

            def p2_pair_ns(t):
                """Approx PE-ns of one head's p2_chunk weave() calls."""
                diag = sum(2.2 * (512 - r * 128) / 2.4 + 250
                           for r in range(4))
                full = 4 * t * (2.2 * 512 / 2.4 + 250)
                return diag + full + 2 * 450 + (2 * 250 if t else 0)

            # ---- emission schedule ----
            # Prologue: all bulk loads on ONE queue in strict first-need
            # order (DMA engines drain roughly in issue order, so a later-
            # needed bulk transfer issued early delays an earlier-needed
            # one).  q cols gate the first chain (~1MB), then x halves,
            # then k cols (needed 2 chains in), then v cols.
            w_sb[0] = wpool.tile([128, ND, 768], BF16, tag="w", name="w_h0")
            wT_r0 = wT.ap()[0].rearrange("(n p) e -> p n e", p=128)
            xt_a = xtp.tile([128, 8, 512], BF16, tag="xta", name="xt_a")
            xt_b = xtp.tile([128, 8, 512], BF16, tag="xtb", name="xt_b")
            nc.gpsimd.dma_start(out=w_sb[0][:, :, 0:128],
                                in_=wT_r0[:, :, 0:128])
            nc.gpsimd.dma_start(out=xt_a[:, 0:4, :],
                                in_=xT_r[:, 0:4, 0:512])
            nc.gpsimd.dma_start(out=w_sb[0][:, :, 128:256],
                                in_=wT_r0[:, :, 128:256])
            nc.gpsimd.dma_start(out=xt_a[:, 4:8, :],
                                in_=xT_r[:, 4:8, 0:512])
            nc.gpsimd.dma_start(out=xt_b, in_=xT_r[:, 8:16, 0:512])
            nc.gpsimd.dma_start(out=w_sb[0][:, :, 256:512],
                                in_=wT_r0[:, :, 256:512])
            nc.gpsimd.dma_start(out=w_sb[0][:, :, 512:768],
                                in_=wT_r0[:, :, 512:768])
            nc.sync.dma_start(out=mask_sb, in_=mask.ap())
            nc.sync.dma_start(out=ones_sb, in_=ones.ap()[:, 0:16])
            nc.sync.dma_start(out=ones_pair, in_=ones8.ap())
            xts0 = (xt_a, xt_b)
            p1_alloc(0)

            # P1(hp0): pure qkv streaming for head pair 0
            xts_a0 = None
            for tci in range(NC_CHUNK):
                xts = xts0 if tci == 0 else dma_x(tci)
                if tci == 2:
                    dma_w(1)      # stream hp1 weights under hp0 compute
                if tci == 3:
                    # x chunk 0 for interleave A: woven p1 units consume it
                    # almost immediately at A start, so land it under P1's
                    # last chunk.
                    xts_a0 = dma_x(0)
                for _, u in p1_units(0, tci, xts):
                    u()

            # Interleave A: P2(hp0) j-steps woven with P1(hp1) units.
            # The last chunk's weaver is not drained: its leftover units
            # spill into interleave B's t=0 (which otherwise has no woven
            # PE work and stalls on ACT).
            p1_alloc(1)
            xts = xts_a0
            carry = None
            for t in range(NC_CHUNK):
                weave = make_weaver(p1_units(1, t, xts))
                if t + 1 < NC_CHUNK:
                    weave.set_total(2 * p2_pair_ns(t))
                else:
                    weave.set_total(2 * (p2_pair_ns(t) + p2_pair_ns(0)))
                p2_chunk(0, t, 0, weave)
                if t + 1 < NC_CHUNK:
                    xts = dma_x(t + 1)
                p2_chunk(0, t, 1, weave)
                if t + 1 < NC_CHUNK:
                    weave.drain()
                else:
                    carry = weave

            # Interleave B: P2(hp1) j-steps woven with P3 proj units.
            # The pending tail must flush before batch t-1's proj units may
            # read attnT, so flush it before each chunk's first j-loop.
            dma_wp()
            for t in range(NC_CHUNK):
                if t >= 1:
                    if carry is not None:
                        carry.drain()
                        carry = None
                    weave = make_weaver(p3_units(t - 1))
                    weave.set_total(2 * p2_pair_ns(t))
                else:
                    weave = carry   # leftover P1(hp1) units from A
                p2_chunk(1, t, 0, weave)
                p2_chunk(1, t, 1, weave)
                if t >= 1:
                    weave.drain()
            for _, u in p3_units(NC_CHUNK - 1, evac_act=True):
                u()

    nc.compile()
    return nc


def _get_compiled():
    global _compiled
    if _compiled is None:
        _compiled = _build()
    return _compiled


def ones8_host():
    o8 = np.zeros((128, 2, 16), dtype=ml_dtypes.float8_e4m3)
    o8[:, :, 0] = 1.0
    return o8


def _shard_inputs(x, W_qkv, W_proj):
    """Build the 8 per-core input maps (host-side transposes/slices)."""
    bf16 = ml_dtypes.bfloat16
    x = np.asarray(x, dtype=np.float32)
    W_qkv = np.asarray(W_qkv, dtype=np.float32)
    W_proj = np.asarray(W_proj, dtype=np.float32)

    mask = np.where(np.arange(128)[None, :] >= np.arange(128)[:, None],
                    np.float32(0.0), np.float32(NEG))  # [tk, tq]

    in_maps = []
    for c in range(N_CORES):
        b, g = divmod(c, HEADS_PER_CORE)
        xT = np.ascontiguousarray(x[b].T).astype(bf16)
        wt = np.empty((2, D, 768), dtype=bf16)
        for hp in range(2):
            rows = []
            for blk in range(3):  # q, k, v row blocks of W_qkv
                h0 = (4 * g + 2 * hp) * DH
                rows.append(W_qkv[blk * D + h0: blk * D + h0 + 2 * DH])
            wt[hp] = np.concatenate(rows, axis=0).T.astype(bf16)
        cols = slice(4 * g * DH, 4 * g * DH + HEADS_PER_CORE * DH)
        wpT = np.ascontiguousarray(W_proj[:, cols].T).astype(bf16)
        ones_arr = np.zeros((128, 128), dtype=bf16)
        ones_arr[:, 0] = 1.0
        in_maps.append({"xT": xT, "wT": wt, "wpT": wpT, "mask": mask,
                        "ones": ones_arr, "ones8": ones8_host()})
    return in_maps


def kernel(x, W_qkv, W_proj, step, trace=False, trace_cores=None):
    nc = _get_compiled()
    in_maps = _shard_inputs(x, W_qkv, W_proj)
    res = run_bass_kernel_spmd(nc, in_maps, list(range(N_CORES)),
                               trace=trace, trace_cores=trace_cores)
    y = np.zeros((B, T, D), dtype=np.float32)
    for c in range(N_CORES):
        y[c // HEADS_PER_CORE] += np.asarray(res.results[c]["y"],
                                             dtype=np.float32)
    kernel.last_exec_time_ns = res.exec_time_ns
    return y
